# revision 4
# baseline (speedup 1.0000x reference)
"""Masked dot-product attention (B=16, Lq=Lk=2048, D=64, fp32) on 8 trn2 cores.

Strategy: data-parallel over batches, 2 batches ("slots") per core. Per batch,
compute S^T = K @ Q^T in 128-key blocks (contraction D=64 on partitions),
exponentiate on ScalarE with the 1/sqrt(D) scale and a per-key-row additive
mask bias folded in, and accumulate O_ext^T = V_ext^T @ P^T in PSUM, where
V_ext has a ones column appended so row 64 of the accumulator is the softmax
denominator. Key blocks entirely beyond a slot's max valid_len are skipped at
program-build time (valid_lens are host-visible); the boundary block is masked
by the bias (-1e5 -> exp == 0). No row-max subtraction: scores are ~N(0,1)
after scaling, so exp never overflows. The host divides by the denominator row
and transposes back.
"""

import sys

sys.path.insert(0, "/opt/trn_rl_repo")

import numpy as np

import concourse.bass as bass
import concourse.mybir as mybir
import concourse.tile as tile
from concourse import bacc
from concourse.bass_utils import run_bass_kernel_spmd

B, LQ, LK, D = 16, 2048, 2048, 64
N_CORES = 8
SLOTS = B // N_CORES  # 2
MASK_BIAS = -1.0e5  # exp(x*scale + MASK_BIAS) underflows to exactly 0.0
SCALE = 1.0 / 8.0  # 1/sqrt(D)

F32 = mybir.dt.float32
F32R = mybir.dt.float32r


def _build_program(nbs: list[int]):
    """One SPMD program for all 8 cores. nbs[s] = number of 128-key blocks
    processed for batch-slot s (uniform across cores)."""
    nc = bacc.Bacc("TRN2", target_bir_lowering=False, debug=False, num_devices=N_CORES)

    qt = nc.dram_tensor("qt", [SLOTS, D, LQ], F32R, kind="ExternalInput").ap()
    kts = [
        nc.dram_tensor(f"kt{s}", [D, nbs[s] * 128], F32R, kind="ExternalInput").ap()
        for s in range(SLOTS)
    ]
    ves = [
        nc.dram_tensor(f"ve{s}", [128, nbs[s] * 65], F32R, kind="ExternalInput").ap()
        for s in range(SLOTS)
    ]
    biases = [
        nc.dram_tensor(f"bias{s}", [128, nbs[s]], F32, kind="ExternalInput").ap()
        for s in range(SLOTS)
    ]
    out = nc.dram_tensor("o", [SLOTS, 65, LQ], F32, kind="ExternalOutput").ap()

    with tile.TileContext(nc) as tc:
        with (
            tc.tile_pool(name="qpool", bufs=1) as qpool,
            tc.tile_pool(name="kpool", bufs=1) as kpool,
            tc.tile_pool(name="vpool", bufs=1) as vpool,
            tc.tile_pool(name="bpool", bufs=1) as bpool,
            tc.tile_pool(name="spsum", bufs=2, space="PSUM") as spool,
            tc.tile_pool(name="opsum", bufs=1, space="PSUM") as opool,
            tc.tile_pool(name="ppool", bufs=3) as ppool,
            tc.tile_pool(name="osb", bufs=2) as opool_sb,
        ):
            for s in range(SLOTS):
                nb = nbs[s]
                qt_sb = qpool.tile([D, LQ], F32R, tag=f"qt{s}")
                # split the Q^T load so compute can start after the first half
                nc.sync.dma_start(out=qt_sb[:, : LQ // 2], in_=qt[s, :, : LQ // 2])
                nc.sync.dma_start(out=qt_sb[:, LQ // 2 :], in_=qt[s, :, LQ // 2 :])
                kt_sb = kpool.tile([D, nb * 128], F32R, tag=f"kt{s}")
                nc.sync.dma_start(out=kt_sb[:], in_=kts[s][:])
                ve_sb = vpool.tile([128, nb * 65], F32R, tag=f"ve{s}")
                nc.sync.dma_start(out=ve_sb[:], in_=ves[s][:])
                bias_sb = bpool.tile([128, nb], F32, tag=f"bias{s}")
                nc.sync.dma_start(out=bias_sb[:], in_=biases[s][:])

                op = opool.tile([65, LQ], F32, tag="opsum")
                for ki in range(nb):
                    kt_blk = kt_sb[:, ki * 128 : (ki + 1) * 128]
                    ve_blk = ve_sb[:, ki * 65 : (ki + 1) * 65]
                    pts = []
                    for qh in range(2):  # halves of the q dim, 1024 each
                        sp = spool.tile([128, LQ // 2], F32, tag="spsum")
                        for qj in range(2):  # 512-wide matmuls (one PSUM bank)
                            q0 = qh * 1024 + qj * 512
                            nc.tensor.matmul(
                                sp[:, qj * 512 : (qj + 1) * 512],
                                lhsT=kt_blk,
                                rhs=qt_sb[:, q0 : q0 + 512],
                                start=True,
                                stop=True,
                            )
                        pt = ppool.tile([128, LQ // 2], F32R, tag="pt")
                        nc.scalar.activation(
                            pt[:],
                            sp[:],
                            mybir.ActivationFunctionType.Exp,
                            bias=bias_sb[:, ki : ki + 1],
                            scale=SCALE,
                        )
                        pts.append(pt)
                    for qh in range(2):
                        for qj in range(2):
                            q0 = qh * 1024 + qj * 512
                            nc.tensor.matmul(
                                op[:, q0 : q0 + 512],
                                lhsT=ve_blk,
                                rhs=pts[qh][:, qj * 512 : (qj + 1) * 512],
                                start=(ki == 0),
                                stop=(ki == nb - 1),
                            )
                o_sb = opool_sb.tile([65, LQ], F32, tag="osb")
                nc.vector.tensor_copy(o_sb[:], op[:])
                nc.sync.dma_start(out=out[s], in_=o_sb[:])

    nc.compile()
    return nc


def _run(queries, keys, values, valid_lens, trace=False):
    queries = np.asarray(queries, dtype=np.float32)
    keys = np.asarray(keys, dtype=np.float32)
    values = np.asarray(values, dtype=np.float32)
    vl = np.asarray(valid_lens).astype(np.int64)
    assert queries.shape == (B, LQ, D), queries.shape

    # Slot assignment: sort batches by valid_len descending; slot s of core c
    # handles batch order[s*8 + c]. Each slot's block count is the max over
    # its 8 batches, so grouping similar lengths minimizes wasted blocks.
    order = np.argsort(-vl, kind="stable")
    nbs = []
    for s in range(SLOTS):
        sb = order[s * N_CORES : (s + 1) * N_CORES]
        nbs.append(max(1, int(-(-int(vl[sb].max()) // 128))))

    nc = _build_program(nbs)

    in_maps = []
    for c in range(N_CORES):
        m = {}
        qt = np.empty((SLOTS, D, LQ), dtype=np.float32)
        for s in range(SLOTS):
            b = int(order[s * N_CORES + c])
            nb = nbs[s]
            nk = nb * 128
            qt[s] = queries[b].T
            m[f"kt{s}"] = np.ascontiguousarray(keys[b, :nk].T)
            ve = np.empty((nk, 65), dtype=np.float32)
            ve[:, :D] = values[b, :nk]
            ve[:, D] = 1.0
            # [128, nb*65] with SBUF partition p holding rows p, 128+p, ...
            m[f"ve{s}"] = np.ascontiguousarray(
                ve.reshape(nb, 128, 65).transpose(1, 0, 2).reshape(128, nb * 65)
            )
            kidx = np.arange(nk).reshape(nb, 128).T  # [128, nb]
            m[f"bias{s}"] = np.where(kidx < vl[b], 0.0, MASK_BIAS).astype(np.float32)
        m["qt"] = qt
        in_maps.append(m)

    res = run_bass_kernel_spmd(nc, in_maps, list(range(N_CORES)), trace=trace)

    out = np.empty((B, LQ, D), dtype=np.float32)
    for c in range(N_CORES):
        o = res.results[c]["o"]  # [SLOTS, 65, LQ]
        for s in range(SLOTS):
            b = int(order[s * N_CORES + c])
            out[b] = (o[s, :D] / o[s, D]).T
    return out, res


def kernel(queries, keys, values, valid_lens):
    out, _ = _run(queries, keys, values, valid_lens)
    return out


def kernel_profiled(queries, keys, values, valid_lens):
    """Returns exec_time_ns; requires the axon NTFF profile hook installed."""
    _, res = _run(queries, keys, values, valid_lens, trace=True)
    if res.instructions_and_trace:
        print("trace:", res.instructions_and_trace[1])
    return res.exec_time_ns


# revision 26
# speedup vs baseline: 2.0622x; 2.0622x over previous
"""Masked dot-product attention (B=16, Lq=Lk=2048, D=64, fp32) on 8 trn2 cores.

Work decomposition: the valid (batch, 128-key-block) space — valid_lens are
host-visible, so key blocks past each batch's valid length are never computed
— is split into contiguous-k "jobs" and packed into an 8-core x J-slot grid
(slot j runs nbs[j] blocks on every core; SPMD requires uniform shape). Jobs
of one batch on different cores produce partial unnormalized outputs that the
host sums — exact, because no row-max is subtracted (scores are ~N(0,1) after
the 1/sqrt(D) scale, so exp cannot overflow).

Per job: S^T = K @ Q^T per key block via PE (contraction D=64 on partitions;
Q^T/K^T are duplicated into partitions 64-127 so paired matmuls run
concurrently on the two 64-row PE array tiles), P^T = exp(S^T*scale + bias)
on ScalarE (bias = 0 valid / -1e5 masked, applied per key row = partition),
then O_ext^T += V_ext^T @ P^T accumulates in PSUM, where V_ext carries a ones
column so row 64 of O_ext^T is the softmax denominator. Host divides and
transposes. Matmul operands are bf16 (PE here is clock-capped at 1.2 GHz;
bf16 mainly cheapens weight loads + halves DMA), accumulation is fp32.
"""

import math
import sys

sys.path.insert(0, "/opt/trn_rl_repo")

import ml_dtypes
import numpy as np

import concourse.mybir as mybir
import concourse.tile as tile
from concourse import bacc
from concourse.bass_utils import run_bass_kernel_spmd

B, LQ, LK, D = 16, 2048, 2048, 64
N_CORES = 8
MASK_BIAS = -1.0e5  # exp(x*scale + MASK_BIAS) underflows to exactly 0.0
SCALE = 1.0 / 8.0  # 1/sqrt(D)

F32 = mybir.dt.float32
BF16 = mybir.dt.bfloat16
MM_DT = BF16
MM_NP = ml_dtypes.bfloat16


# ---------------------------------------------------------------- planning


def _profiles(total, max_part, max_len=5):
    """Descending part lists summing to `total`, parts <= max_part."""
    out = []

    def rec(rem, cap, cur):
        if rem == 0:
            out.append(tuple(cur))
            return
        if len(cur) >= max_len:
            return
        for p in range(min(cap, rem), 0, -1):
            cur.append(p)
            rec(rem - p, p, cur)
            cur.pop()

    rec(total, max_part, [])
    out.sort(key=lambda t: (len(t), -t[0]))
    return out


def _try_pack(w, prof):
    """Greedy: largest remaining batch-chunk into largest free slot position.
    Returns {(core, slot): (batch, k0_block, nreal)} or None."""
    import heapq

    free = []  # (-cap, slot, core)
    for j, cap in enumerate(prof):
        for c in range(N_CORES):
            heapq.heappush(free, (-cap, j, c))
    items = [(-wb, b) for b, wb in enumerate(w)]
    heapq.heapify(items)
    placed = {b: 0 for b in range(len(w))}
    assign = {}
    while items:
        nwb, b = heapq.heappop(items)
        wb = -nwb
        if wb == 0:
            continue
        if not free:
            return None
        ncap, j, c = heapq.heappop(free)
        take = min(wb, -ncap)
        assign[(c, j)] = (b, placed[b], take)
        placed[b] += take
        if wb - take > 0:
            heapq.heappush(items, (-(wb - take), b))
    return assign


def _plan_jobs(vl):
    """Pack per-batch block counts into an 8 x J slot grid minimizing
    per-core blocks + per-slot overhead. Returns (nbs, assign)."""
    w = [max(1, -(-int(v) // 128)) for v in vl]
    total_w = sum(w)
    lo = max(-(-total_w // N_CORES), 1)
    cands = []
    for tot in range(lo, lo + 2 * max(w) + 2):
        cands.extend(_profiles(tot, max(w)))
    # ~0.75 key blocks of cost per extra slot (drain + pipeline bubble)
    cands.sort(key=lambda p: (sum(p) + 0.75 * len(p), len(p)))
    for prof in cands:
        a = _try_pack(w, prof)
        if a is not None:
            # shrink each slot to the largest chunk actually placed in it
            nbs = [
                max(
                    (a[(c, j)][2] for c in range(N_CORES) if (c, j) in a),
                    default=0,
                )
                for j in range(len(prof))
            ]
            keep = [j for j, nb in enumerate(nbs) if nb > 0]
            remap = {j: i for i, j in enumerate(keep)}
            nbs = [nbs[j] for j in keep]
            a = {(c, remap[j]): v for (c, j), v in a.items() if j in keep}
            return nbs, a
    raise RuntimeError("packing failed")


# ---------------------------------------------------------------- device


_PROGRAM_CACHE = {}


def _build_program(nbs):
    """One SPMD program for all 8 cores; slot j processes nbs[j] key blocks."""
    key = tuple(nbs)
    if key in _PROGRAM_CACHE:
        return _PROGRAM_CACHE[key]
    nc = bacc.Bacc("TRN2", target_bir_lowering=False, debug=False, num_devices=N_CORES)
    J = len(nbs)

    # Q^T/K^T duplicated into partitions 64-127 so pairs of QK matmuls run
    # concurrently on the two 64-row PE tiles (64x128 array tiling mode).
    qt = nc.dram_tensor("qt", [J, 2 * D, LQ], MM_DT, kind="ExternalInput").ap()
    kts = [
        nc.dram_tensor(f"kt{s}", [2 * D, nbs[s] * 128], MM_DT, kind="ExternalInput").ap()
        for s in range(J)
    ]
    ves = [
        nc.dram_tensor(f"ve{s}", [128, nbs[s] * 65], MM_DT, kind="ExternalInput").ap()
        for s in range(J)
    ]
    biases = [
        nc.dram_tensor(f"bias{s}", [128, nbs[s]], F32, kind="ExternalInput").ap()
        for s in range(J)
    ]
    out = nc.dram_tensor("o", [J, 65, LQ], F32, kind="ExternalOutput").ap()

    with tile.TileContext(nc) as tc:
        with (
            tc.tile_pool(name="qpool", bufs=1) as qpool,
            tc.tile_pool(name="kpool", bufs=1) as kpool,
            tc.tile_pool(name="vpool", bufs=1) as vpool,
            tc.tile_pool(name="bpool", bufs=1) as bpool,
            tc.tile_pool(name="spsum", bufs=2, space="PSUM") as spool,
            tc.tile_pool(name="opsum", bufs=1, space="PSUM") as opool,
            tc.tile_pool(name="ppool", bufs=4) as ppool,
            tc.tile_pool(name="osb", bufs=2) as opool_sb,
        ):
            # Load every slot's inputs up front (everything fits in SBUF).
            # Input loads must precede all output DMAs in each queue's
            # stream, else a slot's output store blocks the next slot's
            # loads (in-order queues). Issues alternate between the Sync
            # and GpSimd queues (~0.8us serialized issue cost each), most
            # urgent first.
            qt_sbs, kt_sbs, ve_sbs, bias_sbs = [], [], [], []
            for s in range(J):
                nb = nbs[s]
                qt_sbs.append(qpool.tile([2 * D, LQ], MM_DT, tag=f"qt{s}", name=f"qt_sb{s}"))
                kt_sbs.append(kpool.tile([2 * D, nb * 128], MM_DT, tag=f"kt{s}", name=f"kt_sb{s}"))
                ve_sbs.append(vpool.tile([128, nb * 65], MM_DT, tag=f"ve{s}", name=f"ve_sb{s}"))
                bias_sbs.append(bpool.tile([128, nb], F32, tag=f"bias{s}", name=f"bias_sb{s}"))
            # dummy exp: forces the ~1.3us exp ACT-table load to happen
            # during the prologue DMA wait instead of before the first
            # real exp
            warm = bpool.tile([128, 1], F32, name="warm")
            nc.vector.memset(warm[:], 0.0)
            nc.scalar.activation(warm[:], warm[:], mybir.ActivationFunctionType.Exp)
            # slot 0 criticals: first K block + first Q columns + bias
            nc.sync.dma_start(out=kt_sbs[0][:, :128], in_=kts[0][:, :128])
            nc.gpsimd.dma_start(out=qt_sbs[0][:, :512], in_=qt[0, :, :512])
            nc.sync.dma_start(out=bias_sbs[0][:], in_=biases[0][:])
            if nbs[0] > 1:
                nc.sync.dma_start(out=kt_sbs[0][:, 128:], in_=kts[0][:, 128:])
            nc.gpsimd.dma_start(out=qt_sbs[0][:, 512:1024], in_=qt[0, :, 512:1024])
            nc.gpsimd.dma_start(out=ve_sbs[0][:], in_=ves[0][:])
            nc.sync.dma_start(out=qt_sbs[0][:, 1024:], in_=qt[0, :, 1024:])
            for s in range(1, J):
                nc.sync.dma_start(out=kt_sbs[s][:], in_=kts[s][:])
                nc.gpsimd.dma_start(out=qt_sbs[s][:, :1024], in_=qt[s, :, :1024])
                nc.sync.dma_start(out=qt_sbs[s][:, 1024:], in_=qt[s, :, 1024:])
                nc.gpsimd.dma_start(out=ve_sbs[s][:], in_=ves[s][:])
                nc.sync.dma_start(out=bias_sbs[s][:], in_=biases[s][:])

            deferred = []
            for s in range(J):
                nb = nbs[s]
                qt_sb, kt_sb = qt_sbs[s], kt_sbs[s]
                ve_sb, bias_sb = ve_sbs[s], bias_sbs[s]

                op = opool.tile([65, LQ], F32, tag="opsum")
                # software pipeline: PV trails QK/exp by one key block, so
                # the in-order PE queue never stalls on a PV whose exp (or
                # the O accumulator, at slot boundaries) isn't ready yet.
                prev_pts = None
                for ki in range(nb + 1):
                    pts = []
                    if ki < nb:
                        for qh in range(2):  # halves of the q dim, 1024 each
                            sp = spool.tile([128, LQ // 2], F32, tag="spsum")
                            for qj in range(2):  # 512-wide MMs (one bank)
                                q0 = qh * 1024 + qj * 512
                                p0 = qj * D  # alternate 64-row PE tiles
                                nc.tensor.matmul(
                                    sp[:, qj * 512 : (qj + 1) * 512],
                                    lhsT=kt_sb[p0 : p0 + D, ki * 128 : (ki + 1) * 128],
                                    rhs=qt_sb[p0 : p0 + D, q0 : q0 + 512],
                                    start=True,
                                    stop=True,
                                )
                            pt = ppool.tile([128, LQ // 2], MM_DT, tag="pt")
                            nc.scalar.activation(
                                pt[:],
                                sp[:],
                                mybir.ActivationFunctionType.Exp,
                                bias=bias_sb[:, ki : ki + 1],
                                scale=SCALE,
                            )
                            pts.append(pt)
                    if ki > 0:
                        kv = ki - 1
                        ve_blk = ve_sb[:, kv * 65 : (kv + 1) * 65]
                        for qh in range(2):
                            for qj in range(2):
                                q0 = qh * 1024 + qj * 512
                                nc.tensor.matmul(
                                    op[:, q0 : q0 + 512],
                                    lhsT=ve_blk,
                                    rhs=prev_pts[qh][:, qj * 512 : (qj + 1) * 512],
                                    start=(kv == 0),
                                    stop=(kv == nb - 1),
                                )
                    prev_pts = pts
                # drain O in quarters so copies start as soon as the last
                # PV strip lands and overlap the stores. The last two
                # slots' drains are emitted after all compute: by then the
                # Scalar queue has no exps left to block, so their copies
                # can split across Vector+Scalar to halve the exposed
                # end-of-kernel drain cascade.
                if s < J - 2:
                    o_sb = opool_sb.tile([65, LQ], F32, tag="osb", name=f"o_sb{s}")
                    for qq in range(4):
                        sl = slice(qq * 512, (qq + 1) * 512)
                        nc.vector.tensor_copy(o_sb[:, sl], op[:, sl])
                        nc.sync.dma_start(out=out[s, :, sl], in_=o_sb[:, sl])
                else:
                    deferred.append((s, op))

            for s, op in deferred:
                o_sb = opool_sb.tile([65, LQ], F32, tag="osb", name=f"o_sb{s}")
                for qq in range(4):
                    sl = slice(qq * 512, (qq + 1) * 512)
                    if qq % 2 == 1:
                        nc.scalar.copy(o_sb[:, sl], op[:, sl])
                    else:
                        nc.vector.tensor_copy(o_sb[:, sl], op[:, sl])
                    nc.sync.dma_start(out=out[s, :, sl], in_=o_sb[:, sl])

    nc.compile()
    _PROGRAM_CACHE[key] = nc
    return nc


# ---------------------------------------------------------------- host


def _run(queries, keys, values, valid_lens, trace=False):
    queries = np.asarray(queries, dtype=np.float32)
    keys = np.asarray(keys, dtype=np.float32)
    values = np.asarray(values, dtype=np.float32)
    vl = np.asarray(valid_lens).astype(np.int64)
    assert queries.shape == (B, LQ, D), queries.shape

    nbs, assign = _plan_jobs(vl)
    J = len(nbs)
    nc = _build_program(nbs)

    qts = {}  # batch -> duplicated Q^T, built once
    for b in range(B):
        q = np.empty((2 * D, LQ), dtype=MM_NP)
        q[:D] = queries[b].T
        q[D:] = q[:D]
        qts[b] = q

    in_maps = []
    for c in range(N_CORES):
        m = {}
        qt = np.zeros((J, 2 * D, LQ), dtype=MM_NP)
        for s in range(J):
            nb = nbs[s]
            nk = nb * 128
            kt = np.zeros((2 * D, nk), dtype=MM_NP)
            ve = np.zeros((nk, 65), dtype=np.float32)
            bias = np.full((128, nb), MASK_BIAS, dtype=np.float32)
            if (c, s) in assign:
                b, k0b, nreal = assign[(c, s)]
                r0, r1 = k0b * 128, min((k0b + nreal) * 128, LK)
                nr = r1 - r0
                qt[s] = qts[b]
                kt[:D, :nr] = keys[b, r0:r1].T
                kt[D:, :nr] = kt[:D, :nr]
                ve[:nr, :D] = values[b, r0:r1]
                ve[:nr, D] = 1.0
                kidx = (r0 + np.arange(nk)).reshape(nb, 128).T  # [128, nb]
                bias = np.where(
                    (kidx < vl[b]) & (kidx < r1), 0.0, MASK_BIAS
                ).astype(np.float32)
            m[f"kt{s}"] = kt
            m[f"ve{s}"] = np.ascontiguousarray(
                ve.reshape(nb, 128, 65).transpose(1, 0, 2).reshape(128, nb * 65)
            ).astype(MM_NP)
            m[f"bias{s}"] = bias
        m["qt"] = qt
        in_maps.append(m)

    res = run_bass_kernel_spmd(nc, in_maps, list(range(N_CORES)), trace=trace)

    acc = np.zeros((B, 65, LQ), dtype=np.float64)
    for c in range(N_CORES):
        o = res.results[c]["o"]  # [J, 65, LQ]
        for s in range(J):
            if (c, s) in assign:
                b, _, _ = assign[(c, s)]
                acc[b] += o[s]
    out = (acc[:, :D] / acc[:, D:]).transpose(0, 2, 1).astype(np.float32)
    return np.ascontiguousarray(out), res


def kernel(queries, keys, values, valid_lens):
    out, _ = _run(queries, keys, values, valid_lens)
    return out


def kernel_profiled(queries, keys, values, valid_lens):
    """Returns exec_time_ns; requires the axon NTFF profile hook installed."""
    _, res = _run(queries, keys, values, valid_lens, trace=True)
    if res.instructions_and_trace:
        print("trace:", res.instructions_and_trace[1])
    return res.exec_time_ns
